# revision 29
# baseline (speedup 1.0000x reference)
"""Trainium2 Bass kernel for a dense cross-attention transformer block.

Reference computation (per batch b):
    xn = LN(x[b]); yn = LN(y[b])
    q = xn@Wq; k = yn@Wk; v = yn@Wv
    a = softmax(mask(q@k^T/sqrt(L)))
    x2 = xn + a@v; x3 = LN(x2)
    out1 = x3 + relu(x3@Win)@Wout
    returns (out1, yn)

Sharding: 8 cores = 4 batches x 2 halves. Core (b, h) owns query rows
[h*1024, (h+1)*1024) and key rows [h*1024, (h+1)*1024) of batch b. It
computes LN(y)/kT/v only for its own key half, then AllGathers kT and v
with its pair core; the gather overlaps LN(x)/qT. Phase A reads the
gathered kT in global key order (slab r = half r), so no per-core
reordering is needed anywhere. All heavy matmuls run in bf16 (f32 PSUM
accumulation); LN/softmax statistics are f32.
"""

import numpy as np
import sys

for _p in ("/opt/trn_rl_repo",):
    if _p not in sys.path:
        sys.path.insert(0, _p)

import concourse.bass as bass
import concourse.bacc as bacc
import concourse.mybir as mybir
import concourse.tile as tile
from concourse.masks import make_identity

P = 128
E = 1024          # embedding dim
L = 4096          # latent dim
SK = 2048         # key rows per batch
SKH = 1024        # key rows per core (own half)
SQH = 1024        # query rows per core (half batch)
B = 4
NCORES = 8
EC = E // P       # 8  e-chunks
LC = L // P       # 32 l-chunks
KC = SK // P      # 16 key tiles full
KCH = SKH // P    # 8  key tiles own half
QT = SQH // P     # 8  q-tiles per core
NEG = -1.0e30
INV_SQRT_L = 1.0 / 64.0

F32 = mybir.dt.float32
BF16 = mybir.dt.bfloat16
I32 = mybir.dt.int32

AF = mybir.ActivationFunctionType
OP = mybir.AluOpType

_CACHE = {}


def _layernorm_tile(nc, pool, out_ap, in_ap, eps_tile):
    """LN over the free dim (1024) of a [128, 1024] f32 tile."""
    stats = pool.tile([P, 2, 6], F32, tag="ln_stats")
    mv = pool.tile([P, 2], F32, tag="ln_mv")
    xr = in_ap.rearrange("p (s d) -> p s d", s=2)
    for s in range(2):
        nc.vector.bn_stats(out=stats[:, s, :], in_=xr[:, s, :])
    nc.vector.bn_aggr(out=mv[:], in_=stats[:])
    sd = pool.tile([P, 1], F32, tag="ln_sd")
    nc.scalar.activation(out=sd[:], in_=mv[:, 1:2], func=AF.Sqrt, bias=eps_tile[:])
    rs = pool.tile([P, 1], F32, tag="ln_rs")
    nc.vector.reciprocal(out=rs[:], in_=sd[:])
    nc.vector.tensor_scalar(
        out=out_ap, in0=in_ap, scalar1=mv[:, 0:1], scalar2=rs[:],
        op0=OP.subtract, op1=OP.mult,
    )



def _copy(eng, out, in_):
    if hasattr(eng, "tensor_copy"):
        eng.tensor_copy(out=out, in_=in_)
    else:
        eng.copy(out=out, in_=in_)

def _ln_stats(nc, pool, in_ap, eps_tile):
    """mean + 1/sigma scalars for a [128, E] f32 tile."""
    stats = pool.tile([P, 2, 6], F32, tag="ln_stats")
    mv = pool.tile([P, 2], F32, tag="ln_mv")
    xr = in_ap.rearrange("p (s d) -> p s d", s=2)
    for s in range(2):
        nc.vector.bn_stats(out=stats[:, s, :], in_=xr[:, s, :])
    nc.vector.bn_aggr(out=mv[:], in_=stats[:])
    sd = pool.tile([P, 1], F32, tag="ln_sd")
    nc.scalar.activation(out=sd[:], in_=mv[:, 1:2], func=AF.Sqrt,
                         bias=eps_tile[:])
    rs = pool.tile([P, 1], F32, tag="ln_rs")
    nc.vector.reciprocal(out=rs[:], in_=sd[:])
    return mv, rs


def _ln_transpose_tile(nc, tc, pools, src_t, row0, out_spill, nT, eps_t,
                       ident, spill_eng):
    """DMA a [128, E] row tile, LN it, spill f32, transpose into nT."""
    p_in, p_tmp, p_bf, p_tp = pools
    t_in = p_in.tile([P, E], F32, tag="ln_in")
    nc.scalar.dma_start(out=t_in[:], in_=src_t.ap()[row0:row0 + P, :])
    mv, rs = _ln_stats(nc, p_tmp, t_in[:], eps_t)
    t_n = p_in.tile([P, E], F32, tag="ln_out")
    nc.vector.tensor_scalar(
        out=t_n[:], in0=t_in[:], scalar1=mv[:, 0:1], scalar2=rs[:],
        op0=OP.subtract, op1=OP.mult)
    spill_eng.dma_start(out=out_spill.ap()[row0:row0 + P, :], in_=t_n[:])
    t_bf = p_bf.tile([P, E], BF16, tag="ln_bf")
    nc.vector.tensor_scalar(
        out=t_bf[:], in0=t_in[:], scalar1=mv[:, 0:1], scalar2=rs[:],
        op0=OP.subtract, op1=OP.mult)
    t = row0 // P
    for ec in range(EC):
        ps = p_tp.tile([P, P], BF16, tag="tp")
        nc.tensor.transpose(ps[:], t_bf[:, ec * P:(ec + 1) * P], ident[:])
        nc.scalar.copy(out=nT[:, ec, t * P:(t + 1) * P], in_=ps[:])


def _wT_matmul(nc, tc, W, actT, ncols, out_rows=None, qT=None, hT=None,
               group_hook=None, post_group=None, pools=None):
    """For lt in [0, 32): out[lt*128:(lt+1)*128, :] = W[:, lsl].T @ act.

    W: [E, L] f32 DRAM, loaded in contiguous [p, ec, 256] chunks (1KB
    lines) on the gpsimd queue, cast to bf16 on gpsimd. actT: [P, EC,
    ncols] bf16 SBUF (moving operand). Groups of 2 l-tiles with 4+4
    PSUM bank ping-pong so evacuation of group g overlaps matmuls of
    g+1. Output: DRAM row-half tensors (out_rows=[r0, r1], each
    [L/2, ncols]), SBUF qT tile, or SBUF hT tile via Relu.
    """
    w_r = W.ap()
    nchunk = ncols // 512
    import contextlib
    ctx = contextlib.ExitStack()
    with ctx:
        if pools is None:
            wt_w = ctx.enter_context(tc.tile_pool(name="wt_w", bufs=2))
            wt_wb = ctx.enter_context(tc.tile_pool(name="wt_wb", bufs=2))
            wt_o = ctx.enter_context(tc.tile_pool(name="wt_o", bufs=6))
        else:
            wt_w, wt_wb, wt_o = pools
        wt_psA = ctx.enter_context(
            tc.tile_pool(name="wt_psA", bufs=4, space="PSUM"))
        wt_psB = ctx.enter_context(
            tc.tile_pool(name="wt_psB", bufs=4, space="PSUM"))
        for g in range(LC // 2):          # 16 groups of 2 lt
            csl = slice(g * 256, (g + 1) * 256)
            wf = wt_w.tile([P, EC, 256], F32, name="wf", tag="wf")
            nc.gpsimd.dma_start(
                out=wf[:], in_=w_r[:, csl].rearrange("(c p) l -> p c l", p=P))
            wb = wt_wb.tile([P, EC, 256], BF16, name="wb", tag="wb")
            nc.gpsimd.tensor_copy(out=wb[:], in_=wf[:])
            if group_hook is not None:
                group_hook(g)
            pool = wt_psA if g % 2 == 0 else wt_psB
            pss = [pool.tile([P, 512], F32, name=f"ps{i}", tag="ps")
                   for i in range(2 * nchunk)]
            for ec in range(EC):
                for lt2 in range(2):
                    for kc in range(nchunk):
                        nc.tensor.matmul(
                            pss[lt2 * nchunk + kc][:],
                            wb[:, ec, lt2 * P:(lt2 + 1) * P],
                            actT[:, ec, kc * 512:(kc + 1) * 512],
                            start=(ec == 0), stop=(ec == EC - 1))
            for lt2 in range(2):
                lt = g * 2 + lt2
                for kc in range(nchunk):
                    ps = pss[lt2 * nchunk + kc]
                    if hT is not None:
                        nc.scalar.activation(
                            out=hT[:, lt, kc * 512:(kc + 1) * 512], in_=ps[:],
                            func=AF.Relu)
                    elif qT is not None:
                        eng = nc.scalar if (lt2 + kc) % 2 == 0 else nc.vector
                        _copy(eng, qT[:, lt, kc * 512:(kc + 1) * 512], ps[:])
                    else:
                        ob = wt_o.tile([P, 512], BF16, name="ob", tag="ob")
                        nc.scalar.copy(out=ob[:], in_=ps[:])
                        rh, rl = lt // 16, lt % 16
                        nc.scalar.dma_start(
                            out=out_rows[rh * 2 + kc].ap()[
                                :, rl * 512:(rl + 1) * 512],
                            in_=ob[:])
            if post_group is not None:
                post_group(g)


def _phase_y(nc, tc, y_h, x_h, Wk, Wv, yn_out, xn_d, kt_b, v_b, ynT, xnT,
             ident, eps_t, emit_ag_r0):
    """LN(y|x interleaved) -> ynT/xnT; v = yn@Wv -> v_b; kT -> kt_b."""
    wv_r = Wv.ap().rearrange("(c p) e -> p c e", p=P)
    with tc.tile_pool(name="wky_w", bufs=2) as wt_w, \
         tc.tile_pool(name="wky_wb", bufs=2) as wt_wb, \
         tc.tile_pool(name="wky_o", bufs=6) as wt_o:
      with tc.tile_pool(name="py_in", bufs=2) as py_in, \
           tc.tile_pool(name="py_tmp", bufs=8) as py_tmp, \
           tc.tile_pool(name="py_bf", bufs=2) as py_bf, \
           tc.tile_pool(name="py_tp", bufs=3, space="PSUM") as py_tp, \
           tc.tile_pool(name="py_wv", bufs=1) as py_wv, \
           tc.tile_pool(name="py_wvs", bufs=1) as py_wvs, \
           tc.tile_pool(name="py_o", bufs=4) as py_o, \
           tc.tile_pool(name="py_ps", bufs=4, space="PSUM") as py_ps:

        wv_b = py_wv.tile([P, EC, E], BF16)       # 2 MB resident

        def emit_v(t):
            for eo in range(E // 512):
                ps = py_ps.tile([P, 512], F32, name="vps", tag="vmm")
                for ec in range(EC):
                    nc.tensor.matmul(
                        ps[:], ynT[:, ec, t * P:(t + 1) * P],
                        wv_b[:, ec, eo * 512:(eo + 1) * 512],
                        start=(ec == 0), stop=(ec == EC - 1))
                vbf = py_o.tile([P, 512], BF16, name="vbf", tag="vbf")
                nc.vector.tensor_copy(out=vbf[:], in_=ps[:])
                nc.sync.dma_start(
                    out=v_b.ap()[t * P:(t + 1) * P, eo * 512:(eo + 1) * 512],
                    in_=vbf[:])

        pools = (py_in, py_tmp, py_bf, py_tp)
        for t in range(KCH):
            _ln_transpose_tile(nc, tc, pools, y_h, t * P, yn_out, ynT,
                               eps_t, ident, nc.sync)
            _ln_transpose_tile(nc, tc, pools, x_h, t * P, xn_d, xnT,
                               eps_t, ident, nc.scalar)
            if t == 0:
                # emitted after the first LN pair so their loads reach
                # the queues first
                for wc in range(EC):
                    wvc = py_wvs.tile([P, 1, E], F32, name="wvc",
                                      tag="wvc")
                    nc.gpsimd.dma_start(
                        out=wvc[:], in_=wv_r[:, wc:wc + 1, :])
                    nc.gpsimd.tensor_copy(
                        out=wv_b[:, wc:wc + 1, :], in_=wvc[:])
            if t >= 2:
                emit_v(t - 2)     # lag keeps PE fed while LN streams
        emit_v(KCH - 2)
        emit_v(KCH - 1)

      wt_pools = (wt_w, wt_wb, wt_o)

      # kT own half; the first two AllGathers (l-half 0) kick after g7
      def post_group(g):
          if g == 7:
              emit_ag_r0()
      _wT_matmul(nc, tc, Wk, ynT, SKH, out_rows=kt_b, pools=wt_pools,
                 post_group=post_group)


def _phase_a(nc, tc, mask_h, ktf, qT, S, riall, ktA, ktB, kt0):
    """Scores + mask + softmax over gathered kT (global key order).

    kb block j (j=0..3) covers global keys [j*512, (j+1)*512): slab
    j//2 of the pair, column half j%2 -> ktf[j%2] rows slab*L..+L.
    Block 0 arrives prefetched in kt0.
    """
    with tc.tile_pool(name="pa_mi", bufs=3) as pa_mi, \
         tc.tile_pool(name="pa_mf", bufs=3) as pa_mf, \
         tc.tile_pool(name="pa_sm", bufs=4) as pa_sm, \
         tc.tile_pool(name="pa_ps", bufs=3, space="PSUM") as pa_ps:

        for j in range(4):
            slab, colh = j // 2, j % 2
            if j == 0:
                kt = kt0
            else:
                kt = (ktA if j % 2 == 0 else ktB).tile(
                    [P, LC, 512], BF16, name="kt", tag="kt")
                for rh in range(2):
                    nc.sync.dma_start(
                        out=kt[:, rh * 16:(rh + 1) * 16, :],
                        in_=ktf[rh * 2 + colh].ap()[
                            slab * P:(slab + 1) * P, :].rearrange(
                            "p (c k) -> p c k", k=512))
            kcol0 = j * 512
            for qt in range(QT):
                ps = pa_ps.tile([P, 512], F32, tag="s")
                for lc in range(LC):
                    nc.tensor.matmul(
                        ps[:], qT[:, lc, qt * P:(qt + 1) * P],
                        kt[:, lc, :],
                        start=(lc == 0), stop=(lc == LC - 1))
                mi = pa_mi.tile([P, 512], I32, tag="mi")
                nc.scalar.dma_start(
                    out=mi[:],
                    in_=mask_h.ap()[qt * P:(qt + 1) * P, kcol0:kcol0 + 512])
                mf = pa_mf.tile([P, 512], F32, tag="mf")
                nc.vector.tensor_scalar_mul(out=mf[:], in0=mi[:], scalar1=NEG)
                nc.vector.tensor_add(
                    out=S[:, qt, kcol0:kcol0 + 512], in0=ps[:], in1=mf[:])

        for qt in range(QT):
            m = pa_sm.tile([P, 1], F32, tag="m")
            nc.vector.reduce_max(
                out=m[:], in_=S[:, qt, :], axis=mybir.AxisListType.X)
            nm = pa_sm.tile([P, 1], F32, tag="nm")
            nc.vector.tensor_scalar_mul(out=nm[:], in0=m[:], scalar1=-INV_SQRT_L)
            rs = pa_sm.tile([P, 1], F32, tag="rs")
            nc.scalar.activation(
                out=S[:, qt, :], in_=S[:, qt, :], func=AF.Exp,
                bias=nm[:], scale=INV_SQRT_L, accum_out=rs[:])
            nc.vector.reciprocal(out=riall[:, qt:qt + 1], in_=rs[:])


def _phase_b(nc, tc, v_full, xn_d, x3_d, x3T, S, riall, ident, eps_t):
    """attn out (P@V) + residual + LN3 -> x3 spill + x3T (SBUF)."""
    with tc.tile_pool(name="pb_v", bufs=1) as pb_v, \
         tc.tile_pool(name="pb_pt", bufs=QT * KC) as pb_pt, \
         tc.tile_pool(name="pb_x", bufs=3) as pb_x, \
         tc.tile_pool(name="pb_tmp", bufs=4) as pb_tmp, \
         tc.tile_pool(name="pb_ptps", bufs=4, space="PSUM") as pb_ptps, \
         tc.tile_pool(name="pb_ps", bufs=4, space="PSUM") as pb_ps:

        v_sb = pb_v.tile([P, KC, E], BF16)       # 4 MB, global key order
        nc.scalar.dma_start(
            out=v_sb[:], in_=v_full.ap().rearrange("(c p) e -> p c e", p=P))

        all_pts = []
        for qt in range(QT):
            for kc in range(KC):
                pps = pb_ptps.tile([P, P], BF16, tag="ptps")
                nc.tensor.transpose(
                    pps[:], S[:, qt, kc * P:(kc + 1) * P], ident[:])
                pt = pb_pt.tile([P, P], BF16, tag="pt")
                nc.vector.tensor_copy(out=pt[:], in_=pps[:])
                all_pts.append(pt)

        x3bs = []

        def x3T_transposes(qt):
            x3b = x3bs[qt]
            for ec in range(EC):
                pps = pb_ptps.tile([P, P], BF16, tag="ptps")
                nc.tensor.transpose(
                    pps[:], x3b[:, ec * P:(ec + 1) * P], ident[:])
                nc.scalar.copy(
                    out=x3T[:, ec, qt * P:(qt + 1) * P], in_=pps[:])

        for qt in range(QT):
            pts = all_pts[qt * KC:(qt + 1) * KC]

            xn_t = pb_x.tile([P, E], F32, tag="xn")
            nc.gpsimd.dma_start(
                out=xn_t[:], in_=xn_d.ap()[qt * P:(qt + 1) * P, :])
            x2 = pb_x.tile([P, E], F32, tag="x2")
            for eo in range(E // 512):
                ps = pb_ps.tile([P, 512], F32, tag="o")
                for kc in range(KC):
                    nc.tensor.matmul(
                        ps[:], pts[kc][:],
                        v_sb[:, kc, eo * 512:(eo + 1) * 512],
                        start=(kc == 0), stop=(kc == KC - 1))
                nc.scalar.activation(
                    out=x2[:, eo * 512:(eo + 1) * 512], in_=ps[:],
                    func=AF.Copy, bias=0.0, scale=riall[:, qt:qt + 1])
            nc.vector.tensor_add(out=x2[:], in0=x2[:], in1=xn_t[:])

            mv, rs = _ln_stats(nc, pb_tmp, x2[:], eps_t)
            x3 = pb_x.tile([P, E], F32, tag="x3")
            nc.vector.tensor_scalar(
                out=x3[:], in0=x2[:], scalar1=mv[:, 0:1], scalar2=rs[:],
                op0=OP.subtract, op1=OP.mult)
            nc.gpsimd.dma_start(
                out=x3_d.ap()[qt * P:(qt + 1) * P, :], in_=x3[:])
            x3b = pb_x.tile([P, E], BF16, tag="x3b")
            nc.vector.tensor_scalar(
                out=x3b[:], in0=x2[:], scalar1=mv[:, 0:1], scalar2=rs[:],
                op0=OP.subtract, op1=OP.mult)
            x3bs.append(x3b)
            if qt >= 1:
                x3T_transposes(qt - 1)   # lag-1: LN3(qt-1) long done
        x3T_transposes(QT - 1)


def _phase_f(nc, tc, Win, Wout, x3_d, x3T, out1):
    """FFN: hT = relu(Win^T @ x3T); out = hT^T @ Wout + x3."""
    wout_r = Wout.ap().rearrange("(c p) e -> p c e", p=P)
    with tc.tile_pool(name="pf_wo", bufs=1) as pf_wo, \
         tc.tile_pool(name="pf_wos", bufs=2) as pf_wos, \
         tc.tile_pool(name="pf_h", bufs=1) as pf_h, \
         tc.tile_pool(name="pf_x", bufs=2) as pf_x, \
         tc.tile_pool(name="pf_o", bufs=3) as pf_o:

        wout_b = pf_wo.tile([P, LC, E], BF16)    # 8 MB resident
        hT = pf_h.tile([P, LC, SQH], BF16)       # 8 MB [l_loc, lc, q]

        # Wout resident loads interleave with the Win group loads (2
        # rows per hT group) so Win g0 is first in the DMA stream.
        def load_wout(g):
            for lt in (2 * g, 2 * g + 1):
                wf = pf_wos.tile([P, E], F32, name="wo_f", tag="wo_f")
                nc.scalar.dma_start(out=wf[:], in_=wout_r[:, lt, :])
                eng = nc.scalar if lt % 2 == 0 else nc.vector
                _copy(eng, wout_b[:, lt, :], wf[:])

        _wT_matmul(nc, tc, Win, x3T, SQH, hT=hT, group_hook=load_wout)

        pf_ps_cm = tc.tile_pool(name="pf_ps", bufs=3, space="PSUM")
        pf_ps = pf_ps_cm.__enter__()
        for qt in range(QT):
            x3_t = pf_x.tile([P, E], F32, tag="x3r")
            nc.gpsimd.dma_start(
                out=x3_t[:], in_=x3_d.ap()[qt * P:(qt + 1) * P, :])
            for eo in range(E // 512):
                ps = pf_ps.tile([P, 512], F32, tag="f")
                for lc in range(LC):
                    nc.tensor.matmul(
                        ps[:], hT[:, lc, qt * P:(qt + 1) * P],
                        wout_b[:, lc, eo * 512:(eo + 1) * 512],
                        start=(lc == 0), stop=(lc == LC - 1))
                o_t = pf_o.tile([P, 512], F32, tag="o")
                nc.vector.tensor_add(
                    out=o_t[:], in0=ps[:],
                    in1=x3_t[:, eo * 512:(eo + 1) * 512])
                nc.sync.dma_start(
                    out=out1.ap()[qt * P:(qt + 1) * P,
                                  eo * 512:(eo + 1) * 512],
                    in_=o_t[:])
        pf_ps_cm.__exit__(None, None, None)


def _allgather(nc, in_t, out_t, sim):
    if sim:
        # timing stand-in so single-core TimelineSim works
        n = in_t.shape[0]
        nc.sync.dma_start(out=out_t.ap()[0:n, :], in_=in_t.ap())
        nc.sync.dma_start(out=out_t.ap()[n:2 * n, :], in_=in_t.ap())
    else:
        nc.gpsimd.collective_compute(
            "AllGather", mybir.AluOpType.bypass,
            replica_groups=[[0, 1], [2, 3], [4, 5], [6, 7]],
            ins=[in_t.ap()], outs=[out_t.ap()],
        )


def _graph(nc, tc, x_h, y_h, mask_h, Wq, Wk, Wv, Win, Wout,
           out1, yn_out, kt_b, ktf, v_b, v_full, xn_d, x3_d, x3T_d, sim):
    with tc.tile_pool(name="consts", bufs=1) as consts:
        ident = consts.tile([P, P], BF16)
        make_identity(nc, ident[:])
        eps_t = consts.tile([P, 1], F32)
        nc.vector.memset(eps_t[:], 1e-5)
        riall = consts.tile([P, QT], F32)   # softmax 1/rowsum, A->B

        with tc.tile_pool(name="S_pool", bufs=1) as S_pool:
            S = S_pool.tile([P, QT, SK], BF16)    # 4 MB [q_loc, qt, k]
            del x3T_d
            with tc.tile_pool(name="xnT_pool", bufs=1) as xnT_pool:
                xnT = xnT_pool.tile([P, EC, SQH], BF16)   # 2 MB
                with tc.tile_pool(name="ynT_pool", bufs=1) as yp:
                    ynT = yp.tile([P, EC, SKH], BF16)
                    def _ag_r0():
                        _allgather(nc, kt_b[0], ktf[0], sim)
                        _allgather(nc, kt_b[1], ktf[1], sim)
                    _phase_y(nc, tc, y_h, x_h, Wk, Wv, yn_out,
                             xn_d, kt_b, v_b, ynT, xnT, ident, eps_t,
                             _ag_r0)
                _allgather(nc, kt_b[2], ktf[2], sim)
                _allgather(nc, kt_b[3], ktf[3], sim)
                _allgather(nc, v_b, v_full, sim)

                with tc.tile_pool(name="qT_pool", bufs=1) as qT_pool, \
                     tc.tile_pool(name="ktA", bufs=1) as ktA, \
                     tc.tile_pool(name="ktB", bufs=1) as ktB:
                    qT = qT_pool.tile([P, LC, SQH], BF16)   # 8 MB
                    # prefetch kb0 (gathered slab0, cols 0:512) during qT
                    kt0 = ktA.tile([P, LC, 512], BF16, name="kt0", tag="kt")
                    for rh in range(2):
                        nc.sync.dma_start(
                            out=kt0[:, rh * 16:(rh + 1) * 16, :],
                            in_=ktf[rh * 2].ap()[0:P, :].rearrange(
                                "p (c k) -> p c k", k=512))
                    _wT_matmul(nc, tc, Wq, xnT, SQH, qT=qT)
                    _phase_a(nc, tc, mask_h, ktf, qT, S, riall,
                             ktA, ktB, kt0)
            # x3T reuses S's SBUF space (same pool, bufs=1): every pt
            # transpose (S's last reads) precedes the first x3T write
            x3T = S_pool.tile([P, EC, SQH], BF16, name="x3T", tag="S")
            _phase_b(nc, tc, v_full, xn_d, x3_d, x3T, S, riall,
                     ident, eps_t)
            _phase_f(nc, tc, Win, Wout, x3_d, x3T, out1)


def _build(sim=False):
    nc = bacc.Bacc("TRN2", target_bir_lowering=False, debug=False,
                   num_devices=1 if sim else NCORES)

    x_h = nc.dram_tensor("x_h", [SQH, E], F32, kind="ExternalInput")
    y_h = nc.dram_tensor("y_h", [SKH, E], F32, kind="ExternalInput")
    mask_h = nc.dram_tensor("mask_h", [SQH, SK], I32, kind="ExternalInput")
    Wq = nc.dram_tensor("Wq", [E, L], F32, kind="ExternalInput")
    Wk = nc.dram_tensor("Wk", [E, L], F32, kind="ExternalInput")
    Wv = nc.dram_tensor("Wv", [E, E], F32, kind="ExternalInput")
    Win = nc.dram_tensor("Win", [E, L], F32, kind="ExternalInput")
    Wout = nc.dram_tensor("Wout", [L, E], F32, kind="ExternalInput")

    out1 = nc.dram_tensor("out1", [SQH, E], F32, kind="ExternalOutput")
    yn_out = nc.dram_tensor("yn_out", [SKH, E], F32, kind="ExternalOutput")

    # collective bounce (inputs) and gathered outputs. kT is stored in
    # SBUF-tile layout [l_loc, (lt_local, k)] split by (l-half, col-half)
    # so phase A reads are fully contiguous (16KB lines) and the first
    # half's gather wire time overlaps the second half of the kT compute
    kt_b = [nc.dram_tensor(f"kt_b{i}", [P, 16 * 512], BF16)
            for i in range(4)]    # index = rh*2 + kc
    ktf = [nc.dram_tensor(f"ktf{i}", [2 * P, 16 * 512], BF16)
           for i in range(4)]
    v_b = nc.dram_tensor("v_b", [SKH, E], BF16)
    v_full = nc.dram_tensor("v_full", [SK, E], BF16)
    xn_d = nc.dram_tensor("xn_d", [SQH, E], F32)
    x3_d = nc.dram_tensor("x3_d", [SQH, E], F32)

    with tile.TileContext(nc) as tc:
        _graph(nc, tc, x_h, y_h, mask_h, Wq, Wk, Wv, Win, Wout,
               out1, yn_out, kt_b, ktf, v_b, v_full, xn_d, x3_d, None,
               sim)
    nc.compile()
    return nc


def _get_compiled(sim=False):
    key = ("v2", sim)
    if key not in _CACHE:
        _CACHE[key] = _build(sim)
    return _CACHE[key]


def _check_trivial(inputs):
    for n in ("ln1_w", "ln2_w", "ln3_w"):
        if n in inputs and not np.allclose(np.asarray(inputs[n]), 1.0):
            raise NotImplementedError(f"nontrivial {n} unsupported")
    for n in ("ln1_b", "ln2_b", "ln3_b", "bq", "bk", "bv", "bin", "bout"):
        if n in inputs and not np.allclose(np.asarray(inputs[n]), 0.0):
            raise NotImplementedError(f"nontrivial {n} unsupported")


LAST_EXEC_NS = None
TRACE = False
_TIMED = {}


def build_timed(nc, in_maps):
    """Jitted 8-core dispatch fn (PJRT shard_map over _bass_exec_p) with
    device-resident concatenated inputs and donated output buffers.

    One executable serves both correctness and timing runs -- loading a
    second collective NEFF in the same process desyncs the device mesh.
    """
    import jax
    from jax.sharding import Mesh, PartitionSpec, NamedSharding
    from jax.experimental.shard_map import shard_map
    from concourse.bass2jax import (
        _bass_exec_p, install_neuronx_cc_hook, partition_id_tensor)

    install_neuronx_cc_hook()
    n_cores = len(in_maps)
    partition_name = (nc.partition_id_tensor.name
                      if nc.partition_id_tensor else None)

    in_names, out_names, out_avals, zero_outs = [], [], [], []
    for alloc in nc.m.functions[0].allocations:
        if not isinstance(alloc, mybir.MemoryLocationSet):
            continue
        name = alloc.memorylocations[0].name
        if alloc.kind == "ExternalInput":
            if name != partition_name:
                in_names.append(name)
        elif alloc.kind == "ExternalOutput":
            out_names.append(name)
            shape = tuple(alloc.tensor_shape)
            dtype = mybir.dt.np(alloc.dtype)
            out_avals.append(jax.core.ShapedArray(shape, dtype))
            zero_outs.append(np.zeros(shape, dtype))
    n_params = len(in_names)
    n_outs = len(out_avals)
    in_names_all = list(in_names) + list(out_names)
    if partition_name is not None:
        in_names_all.append(partition_name)
    donate = tuple(range(n_params, n_params + n_outs))

    def _body(*args):
        operands = list(args)
        if partition_name is not None:
            operands.append(partition_id_tensor())
        outs = _bass_exec_p.bind(
            *operands,
            out_avals=tuple(out_avals),
            in_names=tuple(in_names_all),
            out_names=tuple(out_names),
            lowering_input_output_aliases=(),
            sim_require_finite=True,
            sim_require_nnan=True,
            nc=nc,
        )
        return tuple(outs)

    devices = jax.devices()[:n_cores]
    assert len(devices) == n_cores
    mesh = Mesh(np.asarray(devices), ("core",))
    in_specs = (PartitionSpec("core"),) * (n_params + n_outs)
    out_specs = (PartitionSpec("core"),) * n_outs
    fn = jax.jit(
        shard_map(_body, mesh=mesh, in_specs=in_specs,
                  out_specs=out_specs, check_rep=False),
        donate_argnums=donate, keep_unused=True,
    )
    sh = NamedSharding(mesh, PartitionSpec("core"))
    concat_in = [
        jax.device_put(
            np.concatenate([np.asarray(in_maps[c][nm])
                            for c in range(n_cores)], axis=0), sh)
        for nm in in_names
    ]
    concat_zeros = [
        jax.device_put(np.zeros((n_cores * z.shape[0], *z.shape[1:]),
                                z.dtype), sh)
        for z in zero_outs
    ]
    return fn, concat_in + concat_zeros, out_names, mesh


def _get_timed(in_maps):
    import jax
    nc = _get_compiled()
    if "fn" not in _TIMED:
        fn, dev_args, out_names, mesh = build_timed(nc, in_maps)
        _TIMED.update(fn=fn, out_names=out_names, mesh=mesh,
                      n_in=len(dev_args) - len(out_names),
                      outs=dev_args[len(dev_args) - len(out_names):])
        ins = dev_args[:_TIMED["n_in"]]
    else:
        from jax.sharding import NamedSharding, PartitionSpec
        sh = NamedSharding(_TIMED["mesh"], PartitionSpec("core"))
        # re-stage inputs for a fresh in_maps
        import concourse.mybir as _mb
        nc2 = nc
        names = []
        partition_name = (nc2.partition_id_tensor.name
                          if nc2.partition_id_tensor else None)
        for alloc in nc2.m.functions[0].allocations:
            if not isinstance(alloc, _mb.MemoryLocationSet):
                continue
            name = alloc.memorylocations[0].name
            if alloc.kind == "ExternalInput" and name != partition_name:
                names.append(name)
        ins = [
            jax.device_put(
                np.concatenate([np.asarray(in_maps[c][nm])
                                for c in range(NCORES)], axis=0), sh)
            for nm in names
        ]
    return _TIMED["fn"], ins, _TIMED


def make_in_maps(x, y, mask, Wq, Wk, Wv, Win, Wout):
    in_maps = []
    for c in range(NCORES):
        b, h = c // 2, c % 2
        in_maps.append({
            "x_h": np.ascontiguousarray(x[b, h * SQH:(h + 1) * SQH]),
            "y_h": np.ascontiguousarray(y[b, h * SKH:(h + 1) * SKH]),
            "mask_h": np.ascontiguousarray(mask[b, h * SQH:(h + 1) * SQH]),
            "Wq": Wq, "Wk": Wk, "Wv": Wv, "Win": Win, "Wout": Wout,
        })
    return in_maps


def kernel(**inputs):
    global LAST_EXEC_NS
    _check_trivial(inputs)
    x = np.ascontiguousarray(np.asarray(inputs["x"], dtype=np.float32))
    y = np.ascontiguousarray(np.asarray(inputs["y"], dtype=np.float32))
    mask = np.ascontiguousarray(np.asarray(inputs["mask"], dtype=np.int32))
    Ws = [np.ascontiguousarray(np.asarray(inputs[n], dtype=np.float32))
          for n in ("Wq", "Wk", "Wv", "Win", "Wout")]

    import jax
    in_maps = make_in_maps(x, y, mask, *Ws)
    last_err = None
    for attempt in range(3):
        try:
            fn, ins, T = _get_timed(in_maps)
            r = fn(*ins, *T["outs"])
            jax.block_until_ready(r)
            T["outs"] = list(r)
            break
        except Exception as e:   # transient device/terminal errors
            last_err = e
            import time as _time
            _time.sleep(10)
    else:
        raise last_err
    out_arrs = {nm: np.asarray(r[i]) for i, nm in enumerate(T["out_names"])}
    o1 = np.empty((B, 2 * SQH, E), np.float32)
    yn = np.empty((B, SK, E), np.float32)
    for c in range(NCORES):
        b, h = c // 2, c % 2
        o1[b, h * SQH:(h + 1) * SQH] = out_arrs["out1"].reshape(
            NCORES, SQH, E)[c]
        yn[b, h * SKH:(h + 1) * SKH] = out_arrs["yn_out"].reshape(
            NCORES, SKH, E)[c]
    return o1, yn
